# revision 11
# baseline (speedup 1.0000x reference)
"""APoT quantizer (nn_APoTQuantizer) as a distributed Bass kernel on 8 TRN2 NeuronCores.

Math: out = alpha_pos * Q(clip(x / alpha_pos, -1, 1)) where Q rounds to the nearest
entry of the 243-entry APoT codebook. Every codebook level is a sum of at most two
powers of two: {0, ±2^-a (a<=15), ±(2^-a + 2^-b), 1 <= a < b <= 15, ±1}. Nearest-level
quantization therefore decomposes per element (y = clipped, normalized input) into:
  lead = sign-preserving power-of-two floor of y   (bits: y & 0xFF800000)
  r    = y - lead                                  (exact; residual, same sign as y)
  q    = round-to-nearest-power-of-2 of r          (= pot-floor of r*(4/3))
  out  = alpha*(lead + q)
No codebook gather needed. alpha*(lead+q) is computed on the TensorEngine as two
diag(alpha)-weighted matmuls accumulating into PSUM, so the whole pipeline uses all
five engines: ACT (scale copy, 4/3 copy), GPSIMD (clamp), DVE (mask, sub, mask),
PE (scaled add), DMA in/out — each under the ~93us/core HBM roofline.
"""
import os
import sys

sys.path.insert(0, "/opt/trn_rl_repo")

import numpy as np

from concourse import bacc, bass, mybir
from concourse.bass_utils import run_bass_kernel_spmd
from concourse.tile import TileContext

N_CORES = 8
ROWS, COLS = 4096, 8192
SHARD_ROWS = ROWS // N_CORES          # 512
P = 128                               # SBUF partitions
FREE = SHARD_ROWS // P * COLS         # 32768 free elems per partition
FD = 2048                             # SBUF tile free dim
N_TILES = FREE // FD
MM = 512                              # matmul free dim (one PSUM bank)

MASK_EXP_SIGN = int(np.int32(np.uint32(0xFF800000).view(np.int32)))  # sign+exponent
FOUR_THIRDS = 4.0 / 3.0

_cache = {}


def _build(inv_alpha: float, alpha_pos: float, n_reps: int = 1,
           external_io: bool = True):
    """Build the SPMD graph. With external_io=False, x/out live in internal DRAM
    (garbage contents) and the pipeline is repeated n_reps times — used only for
    wall-clock timing with tiny host<->device transfers."""
    nc = bacc.Bacc()
    f32 = mybir.dt.float32
    i32 = mybir.dt.int32
    if external_io:
        x_t = nc.declare_dram_parameter("x", [SHARD_ROWS, COLS], f32, isOutput=False)
        o_t = nc.declare_dram_parameter("out", [SHARD_ROWS, COLS], f32, isOutput=True)
    else:
        nc.declare_dram_parameter("x", [P, P], f32, isOutput=False)
        o_ext = nc.declare_dram_parameter("out", [P, P], f32, isOutput=True)
        x_t = nc.dram_tensor("x_int", [SHARD_ROWS, COLS], f32)
        o_t = nc.dram_tensor("o_int", [SHARD_ROWS, COLS], f32)

    # partition p <- rows [4p, 4p+4); free dim = the 4 rows concatenated
    x_ap = x_t[:].rearrange("(p a) f -> p (a f)", p=P)
    o_ap = o_t[:].rearrange("(p a) f -> p (a f)", p=P)

    w_dram = nc.inline_tensor(
        (np.eye(P, dtype=np.float32) * np.float32(alpha_pos)), name="alpha_eye"
    )

    AOp = mybir.AluOpType
    Act = mybir.ActivationFunctionType
    with TileContext(nc) as tc:
        with (
            tc.tile_pool(name="wpool", bufs=1) as wpool,
            tc.tile_pool(name="sbuf", bufs=3) as pool,
            tc.tile_pool(name="psum", bufs=8, space="PSUM") as ppool,
        ):
            wt = wpool.tile([P, P], f32, name="wt")
            nc.sync.dma_start(out=wt[:], in_=w_dram[:])
            for i in range(N_TILES * n_reps):
                i = i % N_TILES
                sl = slice(i * FD, (i + 1) * FD)
                ta = pool.tile([P, FD], f32, tag="A")
                tb = pool.tile([P, FD], f32, tag="B")
                tc_ = pool.tile([P, FD], f32, tag="C")
                td = pool.tile([P, FD], f32, tag="D")
                a_f, b_f, c_f, d_f = ta[:], tb[:], tc_[:], td[:]
                b_i = b_f.bitcast(i32)
                c_i = c_f.bitcast(i32)
                d_i = d_f.bitcast(i32)

                # load x
                nc.sync.dma_start(out=a_f, in_=x_ap[:, sl])
                # z = inv_alpha * x                     [ACT]
                nc.scalar.activation(out=b_f, in_=a_f, func=Act.Copy,
                                     scale=float(inv_alpha))
                # y = clamp(z, -1, 1)                   [GPSIMD, in place]
                nc.gpsimd.tensor_scalar(out=b_f, in0=b_f, scalar1=-1.0, scalar2=1.0,
                                        op0=AOp.max, op1=AOp.min)
                # lead = bits(y) & sign|exp mask        [DVE]
                nc.vector.tensor_scalar(out=c_i, in0=b_i, scalar1=MASK_EXP_SIGN,
                                        scalar2=None, op0=AOp.bitwise_and)
                # r = y - lead                          [DVE]
                nc.vector.scalar_tensor_tensor(out=d_f, in0=c_f, scalar=-1.0, in1=b_f,
                                               op0=AOp.mult, op1=AOp.add)
                # t = r * 4/3                           [ACT, in place]
                nc.scalar.activation(out=d_f, in_=d_f, func=Act.Copy,
                                     scale=FOUR_THIRDS)
                # q = bits(t) & sign|exp mask           [DVE, in place]
                nc.vector.tensor_scalar(out=d_i, in0=d_i, scalar1=MASK_EXP_SIGN,
                                        scalar2=None, op0=AOp.bitwise_and)
                # out = alpha*lead + alpha*q            [PE, accumulate in PSUM]
                for c in range(FD // MM):
                    msl = slice(c * MM, (c + 1) * MM)
                    pt = ppool.tile([P, MM], f32, tag="PS")
                    nc.tensor.matmul(pt[:], wt[:], c_f[:, msl],
                                     start=True, stop=False)
                    nc.tensor.matmul(pt[:], wt[:], d_f[:, msl],
                                     start=False, stop=True)
                    # PSUM -> SBUF (DMA cannot read PSUM)   [ACT]
                    nc.scalar.activation(out=a_f[:, msl], in_=pt[:], func=Act.Copy)
                nc.sync.dma_start(out=o_ap[:, sl], in_=a_f)
            if not external_io:
                ft = pool.tile([P, P], f32, tag="A")
                nc.sync.dma_start(out=ft[:], in_=o_t[:P, :P])
                nc.sync.dma_start(out=o_ext[:], in_=ft[:])
    nc.finalize()
    return nc


def kernel(**inputs) -> np.ndarray:
    x = np.ascontiguousarray(np.asarray(inputs["x"], dtype=np.float32))
    alpha = np.float32(np.asarray(inputs["alpha"]).reshape(()))

    alpha_pos = np.float32(np.abs(alpha) + np.float32(1e-5))
    inv_alpha = float(np.float32(1.0) / alpha_pos)

    key = (float(alpha_pos),)
    if key not in _cache:
        _cache[key] = _build(inv_alpha, float(alpha_pos))
    nc = _cache[key]

    shards = np.split(x, N_CORES, axis=0)
    in_maps = [{"x": s} for s in shards]
    res = run_bass_kernel_spmd(nc, in_maps, core_ids=list(range(N_CORES)))
    out = np.concatenate([r["out"] for r in res.results], axis=0)
    return out.astype(np.float32)


# revision 22
# speedup vs baseline: 5.6779x; 5.6779x over previous
"""APoT quantizer (nn_APoTQuantizer) as a distributed Bass kernel on 8 TRN2 NeuronCores.

Math: out = alpha_pos * Q(clip(x / alpha_pos, -1, 1)) where Q rounds to the nearest
entry of the 243-entry APoT codebook. Every codebook level is a sum of at most two
powers of two: {0, ±2^-a (a<=15), ±(2^-a + 2^-b), 1 <= a < b <= 15, ±1}. Nearest-level
quantization therefore decomposes per element (y = clipped, normalized input) into:
  lead = sign-preserving power-of-two floor of y   (bits: y & 0xFF800000)
  r    = y - lead                                  (exact; residual, same sign as y)
  q    = round-to-nearest-power-of-2 of r          (= pot-floor of r*(4/3))
  out  = alpha*(lead + q)
No codebook gather needed. alpha*(lead+q) is computed on the TensorEngine as two
diag(alpha)-weighted float32r matmuls accumulating into PSUM, so the pipeline uses
all five engines: ACT (scale copy, 4/3 copy, PSUM evacuation), GPSIMD (clamp),
DVE (mask, sub, mask), PE (scaled add), DMA in/out.
"""
import os
import sys

sys.path.insert(0, "/opt/trn_rl_repo")

import numpy as np

from concourse import bacc, bass, mybir
from concourse.bass_utils import run_bass_kernel_spmd
from concourse.tile import TileContext

N_CORES = 8
ROWS, COLS = 4096, 8192
SHARD_ROWS = ROWS // N_CORES          # 512
P = 128                               # SBUF partitions
FREE = SHARD_ROWS // P * COLS         # 32768 free elems per partition
FD = 2048                             # SBUF tile free dim
N_TILES = FREE // FD
MM = 512                              # matmul free dim (one PSUM bank)

MASK_EXP_SIGN = int(np.int32(np.uint32(0xFF800000).view(np.int32)))  # sign+exponent
FOUR_THIRDS = 4.0 / 3.0

_cache = {}


def _build(inv_alpha: float, alpha_pos: float, n_reps: int = 1,
           external_io: bool = True):
    """Build the SPMD graph. With external_io=False, x/out live in internal DRAM
    (garbage contents) and the pipeline is repeated n_reps times — used only for
    wall-clock timing with tiny host<->device transfers."""
    nc = bacc.Bacc()
    f32 = mybir.dt.float32
    f32r = mybir.dt.float32r
    i32 = mybir.dt.int32
    if external_io:
        x_t = nc.declare_dram_parameter("x", [SHARD_ROWS, COLS], f32, isOutput=False)
        o_t = nc.declare_dram_parameter("out", [SHARD_ROWS, COLS], f32, isOutput=True)
    else:
        nc.declare_dram_parameter("x", [P, P], f32, isOutput=False)
        o_ext = nc.declare_dram_parameter("out", [P, P], f32, isOutput=True)
        x_t = nc.dram_tensor("x_int", [SHARD_ROWS, COLS], f32)
        o_t = nc.dram_tensor("o_int", [SHARD_ROWS, COLS], f32)

    # partition p <- rows [4p, 4p+4); free dim = the 4 rows concatenated
    x_ap = x_t[:].rearrange("(p a) f -> p (a f)", p=P)
    o_ap = o_t[:].rearrange("(p a) f -> p (a f)", p=P)

    bf16 = mybir.dt.bfloat16
    w_dram = nc.inline_tensor(
        np.eye(P, dtype=np.float32).astype(mybir.dt.np(bf16)), name="eye_bf16"
    )

    AOp = mybir.AluOpType
    Act = mybir.ActivationFunctionType
    with TileContext(nc) as tc:
        with (
            tc.tile_pool(name="wpool", bufs=1) as wpool,
            tc.tile_pool(name="poolA", bufs=4) as poolA,
            tc.tile_pool(name="poolB", bufs=4) as poolB,
            tc.tile_pool(name="psum", bufs=2, space="PSUM") as ppool,
        ):
            wt = wpool.tile([P, P], bf16, name="wt")
            nc.sync.dma_start(out=wt[:], in_=w_dram[:])
            w_r = wt[:]
            for it in range(N_TILES * n_reps):
                i = it % N_TILES
                sl = slice(i * FD, (i + 1) * FD)
                tx = poolA.tile([P, FD], f32, tag="X")
                ta = poolA.tile([P, FD], f32, tag="O")
                tb = poolB.tile([P, FD], f32, tag="B")
                tc_ = poolB.tile([P, FD], f32, tag="C")
                td = poolB.tile([P, FD], f32, tag="D")
                x_f, a_f, b_f, c_f, d_f = tx[:], ta[:], tb[:], tc_[:], td[:]
                b_i = b_f.bitcast(i32)
                c_i = c_f.bitcast(i32)
                d_i = d_f.bitcast(i32)

                # load x
                nc.sync.dma_start(out=x_f, in_=x_ap[:, sl])
                # z = inv_alpha * x                     [ACT]
                nc.scalar.activation(out=b_f, in_=x_f, func=Act.Copy,
                                     scale=float(inv_alpha))
                # y = clamp(z, -1, 1)                   [GPSIMD, in place]
                nc.gpsimd.tensor_scalar(out=b_f, in0=b_f, scalar1=-1.0, scalar2=1.0,
                                        op0=AOp.max, op1=AOp.min)
                # lead = bits(y) & sign|exp mask        [DVE]
                nc.vector.tensor_scalar(out=c_i, in0=b_i, scalar1=MASK_EXP_SIGN,
                                        scalar2=None, op0=AOp.bitwise_and)
                # r = y - lead                          [DVE]
                nc.vector.scalar_tensor_tensor(out=d_f, in0=c_f, scalar=-1.0, in1=b_f,
                                               op0=AOp.mult, op1=AOp.add)
                # t = r * 4/3   [alternate ACT/GPSIMD by tile to balance engines]
                if it % 2 == 0:
                    nc.scalar.activation(out=d_f, in_=d_f, func=Act.Copy,
                                         scale=FOUR_THIRDS)
                else:
                    nc.gpsimd.tensor_scalar(out=d_f, in0=d_f, scalar1=FOUR_THIRDS,
                                            scalar2=None, op0=AOp.mult)
                # q = bits(t) & sign|exp mask           [DVE, in place]
                nc.vector.tensor_scalar(out=d_i, in0=d_i, scalar1=MASK_EXP_SIGN,
                                        scalar2=None, op0=AOp.bitwise_and)
                # out = alpha*lead + alpha*q            [PE, accumulate in PSUM]
                # bf16 views of lead/q: exact for powers of two — the high 16
                # bits of each f32 element (little-endian: odd bf16 slots)
                c_h = c_f.bitcast(bf16)[:, 1::2]
                d_h = d_f.bitcast(bf16)[:, 1::2]
                PS = 2048                              # 4 PSUM banks
                for h in range(FD // PS):
                    hsl = slice(h * PS, (h + 1) * PS)
                    pt = ppool.tile([P, PS], f32, tag="PS")
                    for c in range(PS // MM):
                        msl = slice(h * PS + c * MM, h * PS + (c + 1) * MM)
                        psl = slice(c * MM, (c + 1) * MM)
                        nc.tensor.matmul(pt[:, psl], w_r, c_h[:, msl],
                                         start=True, stop=False)
                        nc.tensor.matmul(pt[:, psl], w_r, d_h[:, msl],
                                         start=False, stop=True)
                    # PSUM -> SBUF with the alpha scale  [ACT]
                    nc.scalar.activation(out=a_f[:, hsl], in_=pt[:], func=Act.Copy,
                                         scale=float(alpha_pos))
                nc.sync.dma_start(out=o_ap[:, sl], in_=a_f)
            if not external_io:
                ft = poolA.tile([P, P], f32, tag="X")
                nc.sync.dma_start(out=ft[:], in_=o_t[:P, :P])
                nc.sync.dma_start(out=o_ext[:], in_=ft[:])
    nc.finalize()
    return nc


def kernel(**inputs) -> np.ndarray:
    x = np.ascontiguousarray(np.asarray(inputs["x"], dtype=np.float32))
    alpha = np.float32(np.asarray(inputs["alpha"]).reshape(()))

    alpha_pos = np.float32(np.abs(alpha) + np.float32(1e-5))
    inv_alpha = float(np.float32(1.0) / alpha_pos)

    key = (float(alpha_pos),)
    if key not in _cache:
        _cache[key] = _build(inv_alpha, float(alpha_pos))
    nc = _cache[key]

    shards = np.split(x, N_CORES, axis=0)
    in_maps = [{"x": s} for s in shards]
    res = run_bass_kernel_spmd(nc, in_maps, core_ids=list(range(N_CORES)))
    out = np.concatenate([r["out"] for r in res.results], axis=0)
    return out.astype(np.float32)
